# revision 41
# baseline (speedup 1.0000x reference)
"""Trainium2 Bass kernel for nn_Aligner (cross-attention aligner).

Math (per batch element i):
    ex      = ix[i] @ W.T + b          # [L, D]
    eother  = iother[i] @ W.T + b      # [L, D]
    align   = softmax(ex @ eother.T)   # [L, L], softmax over last dim
    out[i]  = align @ iother[i]        # [L, D]

Shapes: B=8, L=2048, D=1024, fp32.  Sharding: batch-parallel, one batch
element per NeuronCore (8 cores), W/b replicated.  No collectives.

Key identities/design:
  * align = softmax(ix @ G @ iother^T + row-terms) with G = W^T @ W; for
    b == 0 the row-terms vanish (softmax-invariant per-row shift).  For
    b != 0 the only softmax-relevant extra term is a per-COLUMN addend
    c_m = iother_m . (W^T b), folded in as one extra rank-2 matmul via a
    host-provided selector constant.  G (a pure weight transform) is
    computed host-side and shipped pre-split.
  * Precision: every matmul operand is stored as a bf16 hi part plus an
    fp8e4m3 lo part (lo = x - bf16(x), shipped with a power-of-2 scale).
    Main pass runs in bf16 (1 cyc/row); both cross terms Xh@Yl + Xl@Yh
    run in ONE fp8 perf_mode=DoubleRow pass (0.5 cyc/row) with the
    interleave (Xh*s1, Xl*s2) x (Yl*s2', Yh*s1') chosen so both products
    carry the same power-of-2 scale, removed when merging into the fp32
    logits.  Per-term accuracy ~2^-13; measured logit abs err ~3e-3 rms.
  * ALL transposes (ix, iother, E) go through the DMA XBAR
    (dma_start(transpose=True), bf16): zero PE transpose cycles.
  * Fused single pass over 8 ix-blocks of 256 rows: proj -> align ->
    softmax (exp emits bf16 E) -> E^T via DMA -> out = E@iother in bf16,
    scaled by 1/Z at PSUM eviction.  iother is re-streamed for stage 4
    from a bf16 DRAM copy made during the io prep phase.

Measured (CoreSim cost model): ~415k ns/core; hardware max-scale-relative
error ~2e-3 across all 8 batches (tolerance 2e-2).
"""

import numpy as np

import concourse.bass as bass
import concourse.mybir as mybir
import concourse.tile as tile
from concourse import bacc

P = 128          # partitions
L = 2048         # sequence length
D = 1024         # feature dim
NB = 8           # batch / cores
KC = D // P      # 8 contraction chunks
DG = D // P      # 8 d-groups
M16 = L // P     # 16 m-chunks of 128
NBLK = L // 256  # 8 ix blocks of 256 rows
MC = L // 256    # 8 m-chunks of 256 for align

F32 = mybir.dt.float32
BF16 = mybir.dt.bfloat16
FP8 = mybir.dt.float8e4
DROW = mybir.MatmulPerfMode.DoubleRow
COPYF = mybir.ActivationFunctionType.Copy
EXP = mybir.ActivationFunctionType.Exp
AX = mybir.AxisListType.X

# cross-pass power-of-2 scales (see module docstring)
#   align:  exq8 = (Exh*2^-2, Exl*2^9), yq8 = (Yl*2^9, Yh*2^-2) -> 2^7
#   proj :  Gq   = (Gh*2^2, Gl*2^14),  xq8 = (Xl*2^12, Xh*2^0) -> 2^14
AL_HI, AL_LO, AL_OUT = 0.25, 512.0, 2.0 ** -7
PJ_XLO, PJ_OUT = 4096.0, 2.0 ** -14


def build_program(zero_bias=True):
    nc = bacc.Bacc("TRN2", target_bir_lowering=False, debug=False)

    ix = nc.dram_tensor("ix", [L, D], F32, kind="ExternalInput").ap()
    iother = nc.dram_tensor("iother", [L, D], F32, kind="ExternalInput").ap()
    Gh_in = nc.dram_tensor("Gh", [P, KC, D], BF16, kind="ExternalInput").ap()
    Gq_in = nc.dram_tensor("Gq", [P, KC, 2, D], FP8, kind="ExternalInput").ap()
    out = nc.dram_tensor("out", [L, D], F32, kind="ExternalOutput").ap()
    if not zero_bias:
        # u = W^T b; e01 = selector with rows 0,1 = ones (host constant)
        u_in = nc.dram_tensor("u", [P, KC], F32, kind="ExternalInput").ap()
        e01_in = nc.dram_tensor("e01", [P, P], BF16, kind="ExternalInput").ap()

    # bf16 copy of iother rows, written once in io-prep, streamed as the
    # stage-4 rhs (one [128,1024] tile per m16 chunk per block).
    iob_dram = nc.dram_tensor("iob_scratch", [M16, P, D], BF16).ap()
    if not zero_bias:
        c_dram = nc.dram_tensor("c_scratch", [P, M16], F32).ap()

    import contextlib
    with tile.TileContext(nc) as tc:
        with contextlib.ExitStack() as _stack:
            def _pool(**kw):
                return _stack.enter_context(tc.tile_pool(**kw))
            g_pool = _pool(name="gpool", bufs=1)
            eo_pool = _pool(name="eo", bufs=1)
            xrow_pool = _pool(name="xrow", bufs=2)
            xsplit_pool = _pool(name="xsplit", bufs=2)
            ixT_pool = _pool(name="ixT", bufs=2)
            xq_pool = _pool(name="xq", bufs=1)
            mrg_pool = _pool(name="mrg", bufs=2)
            exT_pool = _pool(name="exT", bufs=2)
            exq_pool = _pool(name="exq", bufs=1)
            small_pool = _pool(name="small", bufs=12)
            pp_pool = _pool(name="pp", bufs=1, space="PSUM")
            io_stack = contextlib.ExitStack()
            iorow_pool = io_stack.enter_context(
                tc.tile_pool(name="iorow", bufs=2))
            iosplit_pool = io_stack.enter_context(
                tc.tile_pool(name="iosplit", bufs=2))
            px_pool = _pool(name="px", bufs=1, space="PSUM")
            ab_pool = _pool(name="ab", bufs=2, space="PSUM")
            ps4_pool = _pool(name="ps4", bufs=1, space="PSUM")
            # ---- resident weights + eo-side operands --------------------
            Gh = g_pool.tile([P, KC, D], BF16, name="Gh")
            Gq = g_pool.tile([P, KC, 2, D], FP8, name="Gq")
            if not zero_bias:
                u_sb = g_pool.tile([P, KC], F32, name="u_sb")
                nc.sync.dma_start(out=u_sb, in_=u_in)
                e01 = g_pool.tile([P, P], BF16, name="e01")
                nc.sync.dma_start(out=e01, in_=e01_in)

            eoT_h = eo_pool.tile([P, KC, L], BF16, name="eoTh")
            yq8 = eo_pool.tile([P, KC, 2, L], FP8, name="yq8")


            # ---- io prep: one m16 chunk of 128 iother rows --------------
            def io_chunk(m16):
                msl = slice(m16 * P, (m16 + 1) * P)
                rows = iorow_pool.tile([P, D], F32, tag="iorow",
                                       name=f"ior{m16}")
                nc.sync.dma_start(out=rows, in_=iother[msl, :])
                iob = iosplit_pool.tile([P, D], BF16, tag="iob",
                                        name=f"iob{m16}")
                nc.vector.tensor_copy(out=iob, in_=rows)
                nc.sync.dma_start(out=iob_dram[m16], in_=iob)
                lob = iosplit_pool.tile([P, D], BF16, tag="lob",
                                        name=f"iol{m16}")
                nc.vector.tensor_sub(out=lob, in0=rows, in1=iob)
                nc.sync.dma_start(out=eoT_h[:, :, msl], in_=iob,
                                  transpose=True)
                loT = iosplit_pool.tile([P, KC, P], BF16, tag="loT",
                                        name=f"loT{m16}")
                nc.sync.dma_start(out=loT, in_=lob, transpose=True)
                nc.scalar.activation(out=yq8[:, :, 0, msl], in_=loT,
                                     func=COPYF, scale=AL_LO)
                nc.scalar.activation(out=yq8[:, :, 1, msl],
                                     in_=eoT_h[:, :, msl], func=COPYF,
                                     scale=AL_HI)
                del m16

            # ---- bias path: c via PE (only when b != 0) -----------------
            def c_compute():
                # c[m] = sum_d ioT[d, m] * u[d]; lhsT = eoT_h chunks,
                # rhs = u column [128,1] per kc. out psum [128(m), 1].
                crow = small_pool.tile([P, M16], F32, tag="crow", bufs=1,
                                       name="crow")
                for m16 in range(M16):
                    msl = slice(m16 * P, (m16 + 1) * P)
                    psc = ab_pool.tile([P, 2, 256], F32, tag="ab",
                                       name=f"psc{m16}")
                    for kc in range(KC):
                        nc.tensor.matmul(psc[:, 0, 0:1], eoT_h[:, kc, msl],
                                         u_sb[:, kc:kc + 1],
                                         start=(kc == 0),
                                         stop=(kc == KC - 1))
                    nc.vector.tensor_copy(out=crow[:, m16:m16 + 1],
                                          in_=psc[:, 0, 0:1])
                nc.sync.dma_start(out=c_dram, in_=crow)

            # ---- ix prep: one block of 256 rows -> ixT_h + xq8 ----------
            def ix_prep(blk):
                ixT_h = ixT_pool.tile([P, KC, 256], BF16, tag="ixTh",
                                      name=f"ixTh{blk}")
                ixT_l = ixT_pool.tile([P, KC, 256], BF16, tag="ixTl",
                                      bufs=1, name=f"ixTl{blk}")
                for sub in range(2):
                    r0 = blk * 256 + sub * P
                    ssl = slice(sub * P, (sub + 1) * P)
                    rows = xrow_pool.tile([P, D], F32, tag="xrow",
                                          name=f"xr{blk}_{sub}")
                    nc.sync.dma_start(out=rows, in_=ix[r0:r0 + P, :])
                    xbf = xsplit_pool.tile([P, D], BF16, tag="xbf",
                                           name=f"xb{blk}_{sub}")
                    nc.vector.tensor_copy(out=xbf, in_=rows)
                    xlo = xsplit_pool.tile([P, D], BF16, tag="xlo",
                                           name=f"xl{blk}_{sub}")
                    nc.vector.tensor_sub(out=xlo, in0=rows, in1=xbf)
                    nc.sync.dma_start(out=ixT_h[:, :, ssl], in_=xbf,
                                      transpose=True)
                    nc.sync.dma_start(out=ixT_l[:, :, ssl], in_=xlo,
                                      transpose=True)
                xq8 = xq_pool.tile([P, KC, 2, 256], FP8, tag="xq8",
                                   name=f"xq{blk}")
                nc.scalar.activation(out=xq8[:, :, 0, :], in_=ixT_l,
                                     func=COPYF, scale=PJ_XLO)
                nc.scalar.activation(out=xq8[:, :, 1, :], in_=ixT_h,
                                     func=COPYF, scale=1.0)
                return ixT_h, xq8

            # ---- proj: exT_h/exq8 for one block -------------------------
            def proj(blk, ixT_h, xq8):
                exT_h = exT_pool.tile([P, KC, 256], BF16, tag="exTh",
                                      name=f"exTh{blk}")
                exl_b = exT_pool.tile([P, KC, 256], BF16, tag="exl",
                                      bufs=1, name=f"exl{blk}")
                for dgh in range(2):
                    dgs = slice(dgh * 4, (dgh + 1) * 4)
                    pp = pp_pool.tile([P, 4, 256], F32, tag="pp",
                                      name=f"pp{blk}_{dgh}")
                    px = px_pool.tile([P, 4, 256], F32, tag="px",
                                      name=f"px{blk}_{dgh}")
                    for j in range(4):
                        dg = dgh * 4 + j
                        dsl = slice(dg * P, (dg + 1) * P)
                        for kc in range(KC):
                            nc.tensor.matmul(pp[:, j, :], Gh[:, kc, dsl],
                                             ixT_h[:, kc, :],
                                             start=(kc == 0),
                                             stop=(kc == KC - 1))
                    for j in range(4):
                        dg = dgh * 4 + j
                        dsl = slice(dg * P, (dg + 1) * P)
                        for kc in range(KC):
                            nc.tensor.matmul(px[:, j, :], Gq[:, kc, :, dsl],
                                             xq8[:, kc, :, :],
                                             start=(kc == 0),
                                             stop=(kc == KC - 1),
                                             perf_mode=DROW)
                    c32 = mrg_pool.tile([P, 4, 256], F32, tag="c32",
                                        bufs=1, name=f"pc{blk}_{dgh}")
                    nc.scalar.activation(out=c32, in_=px, func=COPYF,
                                         scale=PJ_OUT)
                    t1 = mrg_pool.tile([P, 4, 256], F32, tag="t1",
                                       bufs=1, name=f"pt{blk}_{dgh}")
                    nc.vector.tensor_add(out=t1, in0=pp, in1=c32)
                    nc.scalar.copy(out=exT_h[:, dgs, :], in_=t1)
                    nc.vector.tensor_sub(out=exl_b[:, dgs, :], in0=t1,
                                         in1=exT_h[:, dgs, :])
                exq8 = exq_pool.tile([P, KC, 2, 256], FP8, tag="exq8",
                                     name=f"exq{blk}")
                nc.scalar.activation(out=exq8[:, :, 0, :], in_=exT_h,
                                     func=COPYF, scale=AL_HI)
                nc.scalar.activation(out=exq8[:, :, 1, :], in_=exl_b,
                                     func=COPYF, scale=AL_LO)
                return exT_h, exq8

            # ---- align + softmax for one block --------------------------
            def align_softmax(blk, exT_h, exq8, crow_b=None):
                Es = [E_pool.tile([P, L], F32, tag="E", bufs=3,
                                  name=f"E{blk}_{sub}") for sub in range(2)]
                nms = {}
                for mc in range(MC):
                    msl = slice(mc * 256, (mc + 1) * 256)
                    for sub in range(2):
                        ssl = slice(sub * P, (sub + 1) * P)
                        ab = ab_pool.tile([P, 2, 256], F32, tag="ab",
                                          name=f"al{blk}_{mc}_{sub}")
                        n = 0
                        nmm = KC if zero_bias else KC + 1
                        for kc in range(KC):
                            nc.tensor.matmul(ab[:, 0, :], exT_h[:, kc, ssl],
                                             eoT_h[:, kc, msl],
                                             start=(n == 0),
                                             stop=(n == nmm - 1))
                            n += 1
                        if not zero_bias:
                            nc.tensor.matmul(ab[:, 0, :], e01,
                                             crow_b[:, msl],
                                             start=False, stop=True)
                        for kc in range(KC):
                            nc.tensor.matmul(ab[:, 1, :],
                                             exq8[:, kc, :, ssl],
                                             yq8[:, kc, :, msl],
                                             start=(kc == 0),
                                             stop=(kc == KC - 1),
                                             perf_mode=DROW)
                        cE = mrg_pool.tile([P, 256], F32, tag="cE",
                                           name=f"cE{blk}_{mc}_{sub}")
                        nc.scalar.activation(out=cE, in_=ab[:, 1, :],
                                             func=COPYF, scale=AL_OUT)
                        nc.vector.tensor_add(out=Es[sub][:, msl],
                                             in0=ab[:, 0, :], in1=cE)
                        if mc == 3:
                            nms[sub] = small_pool.tile(
                                [P, 1], F32, tag="nm1",
                                name=f"nm1_{blk}_{sub}")
                            nc.vector.reduce_max(nms[sub],
                                                 Es[sub][:, :1024],
                                                 axis=AX, negate=True)
                ebs, rzs = [], []
                for sub in range(2):
                    negM = small_pool.tile([P, 1], F32, tag="negM",
                                           name=f"nm{blk}_{sub}")
                    nc.vector.reduce_max(negM, Es[sub][:, 1024:], axis=AX,
                                         negate=True)
                    nc.vector.tensor_tensor(out=negM, in0=negM,
                                            in1=nms[sub],
                                            op=mybir.AluOpType.min)
                    zsum = small_pool.tile([P, 1], F32, tag="zsum",
                                           name=f"zs{blk}_{sub}")
                    Eb = Eb_pool.tile([P, L], BF16, tag="Eb",
                                      name=f"Eb{blk}_{sub}")
                    nc.scalar.activation(out=Eb, in_=Es[sub], func=EXP,
                                         bias=negM, scale=1.0,
                                         accum_out=zsum)
                    rz = small_pool.tile([P, 1], F32, tag="rz",
                                         name=f"rz{blk}_{sub}")
                    nc.vector.reciprocal(rz, zsum)
                    ebs.append(Eb)
                    rzs.append(rz)
                return ebs, rzs

            # ---- stage 4: out rows = (E @ iother) * rz ------------------
            def stage4(blk, ebs, rzs):
                ets = []
                for sub in range(2):
                    ET = ET_pool.tile([P, M16, P], BF16, tag="ET",
                                      name=f"ET{blk}_{sub}")
                    nc.sync.dma_start(out=ET[:, :8, :],
                                      in_=ebs[sub][:, :1024],
                                      transpose=True)
                    nc.sync.dma_start(out=ET[:, 8:, :],
                                      in_=ebs[sub][:, 1024:],
                                      transpose=True)
                    ets.append(ET)
                for dg in range(2):
                    dsl = slice(dg * 512, (dg + 1) * 512)
                    pss = [ps4_pool.tile([P, 512], F32, tag=f"s4_{sub}",
                                         name=f"s4_{blk}_{dg}_{sub}")
                           for sub in range(2)]
                    for m4 in range(4):
                        rhs = rhs_pool.tile([P, 4, 512], BF16, tag="rhs",
                                            bufs=2,
                                            name=f"rhs{blk}_{dg}_{m4}")
                        nc.sync.dma_start(
                            out=rhs,
                            in_=iob_dram[m4 * 4:(m4 + 1) * 4, :, dsl]
                            .rearrange("g p d -> p g d"))
                        for i in range(4):
                            m16 = m4 * 4 + i
                            for sub in range(2):
                                nc.tensor.matmul(
                                    pss[sub], ets[sub][:, m16, :],
                                    rhs[:, i, :],
                                    start=(m16 == 0),
                                    stop=(m16 == M16 - 1))
                    for sub in range(2):
                        ot = ot_pool.tile([P, 512], F32, tag="ot",
                                          name=f"ot{blk}_{dg}_{sub}")
                        nc.scalar.activation(out=ot, in_=pss[sub],
                                             func=COPYF, scale=rzs[sub])
                        r0 = blk * 256 + sub * P
                        nc.gpsimd.dma_start(out=out[r0:r0 + P, dsl], in_=ot)

            # ---- emission: ix0+Gh first so the PE starts ~13us in;
            # io chunks stream on the DMA device behind them -------------
            crow_b = None
            pre0 = ix_prep(0)
            nc.sync.dma_start(out=Gh, in_=Gh_in)
            io_chunk(0)
            nc.scalar.dma_start(out=Gq, in_=Gq_in)
            io_chunk(1)
            ex0 = proj(0, *pre0)
            for m16 in range(2, 6):
                io_chunk(m16)
            pre1 = ix_prep(1)
            for m16 in range(6, 10):
                io_chunk(m16)
            ex1 = proj(1, *pre1)
            for m16 in range(10, 16):
                io_chunk(m16)
            io_stack.close()
            E_pool = _pool(name="Ep", bufs=1)
            Eb_pool = _pool(name="Eb", bufs=1)
            ET_pool = _pool(name="ETp", bufs=2)
            rhs_pool = _pool(name="rhs4", bufs=3)
            ot_pool = _pool(name="otp", bufs=2)
            if not zero_bias:
                c_compute()
                crow_b = g_pool.tile([P, L], BF16, name="crow_b")
                # rows 0/1 of crow_b = bf16 hi/lo of c; others never read
                # by the selector matmul (e01 has zeros there).  Build via
                # one strided DMA from c_scratch into partition rows 0/1.
                ctmp = small_pool.tile([P, M16], F32, tag="ct", bufs=1,
                                       name="ctmp")
                nc.sync.dma_start(out=ctmp, in_=c_dram)
                # hi/lo split on DVE, then DMA rows into partition 0/1
                ch = small_pool.tile([P, M16], BF16, tag="ch", bufs=1,
                                     name="ch")
                nc.vector.tensor_copy(out=ch, in_=ctmp)
                cl = small_pool.tile([P, M16], BF16, tag="cl", bufs=1,
                                     name="cl")
                nc.vector.tensor_sub(out=cl, in0=ctmp, in1=ch)
                nc.sync.dma_start(
                    out=crow_b[0:1, :],
                    in_=ch.rearrange("p c -> (c p)").rearrange(
                        "(c p) -> x (c p)", x=1))
                nc.sync.dma_start(
                    out=crow_b[1:2, :],
                    in_=cl.rearrange("p c -> (c p)").rearrange(
                        "(c p) -> x (c p)", x=1))

            exs = {0: ex0, 1: ex1}
            for blk in range(NBLK):
                eb, rz = align_softmax(blk, *exs.pop(blk), crow_b=crow_b)
                if blk + 2 < NBLK:
                    pre = ix_prep(blk + 2)
                    exs[blk + 2] = proj(blk + 2, *pre)
                stage4(blk, eb, rz)

    nc.compile()
    return nc


_NC_CACHE = {}


def _get_nc(zero_bias):
    if zero_bias not in _NC_CACHE:
        _NC_CACHE[zero_bias] = build_program(zero_bias)
    return _NC_CACHE[zero_bias]


def host_prep(W, b):
    """Host-side weight preprocessing: G = W^T W split hi/lo + fp8 pair."""
    import ml_dtypes
    bf = ml_dtypes.bfloat16
    f8 = ml_dtypes.float8_e4m3fn

    W64 = W.astype(np.float64)
    G = (W64.T @ W64).astype(np.float32)
    Gh32 = G.astype(bf).astype(np.float32)
    Gl = G - Gh32
    # [k, d] -> [p, kc, d] with k = kc*128 + p
    def dev(a):
        return np.ascontiguousarray(
            a.reshape(KC, P, D).transpose(1, 0, 2))
    Gh_dev = dev(Gh32).astype(bf)
    Gq_dev = np.ascontiguousarray(np.stack(
        [dev(Gh32 * 4.0), dev(Gl * 16384.0)], axis=2)).astype(f8)
    extras = {}
    if not bool(np.all(b == 0.0)):
        u = (W64.T @ b.astype(np.float64)).astype(np.float32)
        extras["u"] = np.ascontiguousarray(
            u.reshape(KC, P).T).astype(np.float32)
        e01 = np.zeros((P, P), dtype=np.float32)
        e01[0, :] = 1.0
        e01[1, :] = 1.0
        extras["e01"] = e01.astype(bf)
    return Gh_dev, Gq_dev, extras


def kernel(ix, iother, W, b):
    """Full-input entry point: shards batch across 8 NeuronCores."""
    from concourse.bass_utils import run_bass_kernel_spmd

    ix = np.ascontiguousarray(np.asarray(ix, dtype=np.float32))
    iother = np.ascontiguousarray(np.asarray(iother, dtype=np.float32))
    W = np.ascontiguousarray(np.asarray(W, dtype=np.float32))
    b = np.ascontiguousarray(np.asarray(b, dtype=np.float32))

    zero_bias = bool(np.all(b == 0.0))
    nc = _get_nc(zero_bias)
    Gh_dev, Gq_dev, extras = host_prep(W, b)
    in_maps = [
        {"ix": ix[i], "iother": iother[i], "Gh": Gh_dev, "Gq": Gq_dev,
         **extras}
        for i in range(NB)
    ]
    res = run_bass_kernel_spmd(nc, in_maps, list(range(NB)))
    outs = [res.results[i]["out"] for i in range(NB)]
    return np.stack(outs, axis=0).astype(np.float32)
